# revision 55
# baseline (speedup 1.0000x reference)
"""Trainium2 Bass kernel for nn_EntailmentSelfAttention (8-core data parallel).

Problem (per batch element n, sentence s):
  q/k/v head projections (shared per-head weights), energy = q @ k.T per head,
  query-position masking, softmax over the QUERY axis, out = attn @ v,
  fc_out: out @ Wo.T + bo.

Design (one batch element n per NeuronCore; S=2 sentences inside):
  - Transposed on-chip layout: head-dim on partitions, sequence on the free
    axis, so the softmax-over-queries reduces along the free axis.
  - q projection folded on the host (yq = q @ Wq^T Wk, compacted+padded);
    v projection folded into fc_out on the host (wcomb); fc bias on host.
  - Masked queries dropped by host compaction; pad columns carry yq = 0 so
    exp(0) = 1, and the denominator subtracts (QP - cnt) (per-core input).
  - Head PAIRS packed into 128 partitions: energy matmuls row-tiled
    (concurrent via tile_position row groups), attn@v matmuls column-tiled
    into halves of one PSUM bank.
  - exp on ScalarE reads 2-tile batches ([128, 2, QP]) from a double-buffered
    2-bank PSUM pool so ScalarE (the bottleneck engine, ~47us of exp) rarely
    starves.
  - rowsums: per pair, a 3-stage tree on the DVE (two bf16 2x-mode
    tensor_tensor halvings + one short 1x reduce) ~30% cheaper than a flat
    reduce; ACC_PAIRS pairs can ride ScalarE's accum_out instead (balance
    knob, currently empty).
  - v-scaling and denominator pad-correction on GpSimd; PSUM->SBUF casts
    for z and fc are paired (two banks per DVE cast) to amortize overhead.
  - Software pipeline: slot p emits energies(p)+exp(p), softmax(p-1),
    attn@v(p-2), fc-interleave(prev sentence, one jt per slot) so engine
    queues stay mostly dependency-free at their heads and the PE stays
    dense; a warm-up matmul burst during the DMA ramp lifts the HAM clock
    gate to full rate before real work arrives.
  - DMA: key/query streams in 2-pair blocks prefetched 2 blocks ahead from
    the sync queue; values sliced per block on the gpsimd queue; the 2MB
    wcomb load is gated behind a data dependency on pair 1's rowsum so the
    round-robin DMA packet scheduler can't starve the pipeline head with it.
"""

import math

import numpy as np

import concourse.bass as bass
import concourse.tile as tile
from concourse import bacc, mybir
from concourse import bass_utils

# problem shapes (hardcoded per the harness contract)
N, S, L, E, H = 8, 2, 512, 1024, 16
D = E // H  # 64
P = 128
NCORES = 8
LC = L // P  # 4 l-chunks
NP = H // 2  # 8 head pairs
SCALE = 1.0 / math.sqrt(float(L))

F32 = mybir.dt.float32
BF16 = mybir.dt.bfloat16

# --- tunables -------------------------------------------------------------
ACC_PAIRS = set()    # per-sentence pairs whose rowsum rides ScalarE accum_out
                     # (measured: any non-empty set regresses — the extra
                     # unbatched-exp time on ScalarE exceeds the DVE relief)
GP_XVS = True        # v-scaling multiply on GpSimd (else DVE)
WARMUP_MM = 16       # dummy matmuls at start to lift HAM to full clock
                     # during the DMA ramp; 16 ~= the 3.4us the HAM needs,
                     # ending just as the first key/query block lands
S0_DUMMY = 0         # dummy matmuls per energy chunk in sentence 0 (they
                     # delay the critical energy path more than the HAM
                     # warmth they buy; keep 0)


def build_kernel_body(tc, outs, ins, QP):
    nc = tc.nc

    def _c(ap):
        return ap if ap.dtype == BF16 else ap.bitcast(BF16)

    xk, yq, xv = _c(ins["xk"]), _c(ins["yq"]), _c(ins["xv"])
    wcomb, padq = _c(ins["wcomb"]), ins["padq"]
    outT = outs["outT"]

    import contextlib

    with contextlib.ExitStack() as ctx:
        ek = ctx.enter_context
        consts = ek(tc.tile_pool(name="consts", bufs=1))
        xvpool = ek(tc.tile_pool(name="xv", bufs=2))
        kqpool = ek(tc.tile_pool(name="kq", bufs=3))
        atpool = ek(tc.tile_pool(name="at", bufs=4))
        rtpool = ek(tc.tile_pool(name="rt", bufs=2))
        xvspool = ek(tc.tile_pool(name="xvs", bufs=3))
        sumpool = ek(tc.tile_pool(name="sums", bufs=3))
        ztpool = ek(tc.tile_pool(name="zt", bufs=2))
        outpool = ek(tc.tile_pool(name="out", bufs=2))
        pp_e = ek(tc.tile_pool(name="pp_e", bufs=2, space="PSUM"))
        pp_zf = ek(tc.tile_pool(name="pp_zf", bufs=2, space="PSUM"))

        # prime the exp ACT table load (overlaps the input DMAs)
        prim = consts.tile([P, 2], F32, tag="prim")
        nc.vector.memset(prim[:, 0:1], 0.0)
        nc.scalar.activation(prim[:, 1:2], prim[:, 0:1],
                             mybir.ActivationFunctionType.Exp)

        # PE warm-up burst: dense dummy matmuls during the DMA ramp lift the
        # HAM clock-gate to 8/8; steady-state gaps stay below the ~3.4us MID
        # window, so the PE then runs at full clock for the whole kernel.
        wsrc = consts.tile([P, QP], BF16, tag="wsrc")
        nc.gpsimd.memset(wsrc[:], 0.0)
        if WARMUP_MM:
            pwu = pp_e.tile([P, 2, 512], F32, tag="ep", name="ep_warm")
            for w in range(WARMUP_MM):
                nc.tensor.matmul(pwu[:, w % 2, 0:QP], wsrc[0:P, 0:P],
                                 wsrc[:], start=True, stop=True)

        padq_sb = consts.tile([P, S], F32, tag="padq")
        wcomb_sb = consts.tile([P, E // P, E], BF16, tag="wcomb")

        xv_sb, ZT, outs_sb = {}, {}, {}
        pfpair = [None]

        def emit_fc_jt(s, jt):
            if jt % 2 == 0:
                pfpair[0] = pp_zf.tile([P, 2, 512], F32, tag="zf",
                                       name=f"pf_{s}_{jt}")
            pf = pfpair[0]
            for eo in range(E // P):
                nc.tensor.matmul(
                    pf[:, jt % 2, 0:QP],
                    wcomb_sb[:, eo, jt * P:(jt + 1) * P],
                    ZT[s][:, eo, :],
                    start=(eo == 0),
                    stop=(eo == E // P - 1),
                )
            if jt % 2 == 1:
                nc.vector.tensor_copy(
                    outs_sb[s][:, jt - 1:jt + 1, :], pf[:, :, 0:QP])
                nc.sync.dma_start(outT[s, :, jt - 1:jt + 1, :],
                                  outs_sb[s][:, jt - 1:jt + 1, :])

        # ---- deferred (software-pipelined) stage emitters ----
        pend_attnv = []
        pend_soft = []
        zpair = {}

        def flush_one(q):
            if q:
                q.pop(0)()

        def make_soft(s, p_, at, rsum, accum_mode):
            holder = {}

            def run():
                if not accum_mode:
                    # 3-stage tree rowsum: the two TT-add stages run at the
                    # DVE's 2x bf16 mode, leaving only a short 1x reduce
                    h1 = QP // 2
                    h2 = QP // 4
                    r1 = rtpool.tile([P, LC, 2, h1], BF16, tag="r1",
                                     name=f"r1{s}_{p_}")
                    nc.vector.tensor_tensor(
                        r1[:], at[:, :, :, 0:h1], at[:, :, :, h1:QP],
                        mybir.AluOpType.add)
                    r2 = rtpool.tile([P, LC, 2, h2], BF16, tag="r2",
                                     name=f"r2{s}_{p_}")
                    nc.vector.tensor_tensor(
                        r2[:], r1[:, :, :, 0:h2], r1[:, :, :, h2:h1],
                        mybir.AluOpType.add)
                    nc.vector.tensor_reduce(
                        rsum[:].rearrange("p (c i) -> p c i", i=2),
                        r2[:],
                        axis=mybir.AxisListType.X,
                        op=mybir.AluOpType.add,
                    )
                den = sumpool.tile([P, LC * 2], F32, tag="den", name=f"dn{s}_{p_}")
                nc.gpsimd.tensor_tensor(
                    den[:], rsum[:],
                    padq_sb[:, s:s + 1].to_broadcast((P, LC * 2)),
                    mybir.AluOpType.subtract)
                recip = sumpool.tile([P, LC, 2], F32, tag="recip", name=f"rc{s}_{p_}")
                nc.vector.reciprocal(recip[:].rearrange("p c i -> p (c i)"), den[:])
                xvs = xvspool.tile([P, LC, 2, D], BF16, tag="xvs", name=f"xs{s}_{p_}")
                xv_view = xv_sb[s][:, :, 2 * p_ * D:(2 * p_ + 2) * D].rearrange(
                    "p c (i d) -> p c i d", d=D)
                eng = nc.gpsimd if GP_XVS else nc.vector
                eng.tensor_tensor(
                    xvs[:], xv_view,
                    recip[:, :, :, None].to_broadcast((P, LC, 2, D)),
                    mybir.AluOpType.mult)
                holder["xvs"] = xvs
            run.holder = holder
            return run

        def make_attnv(s, p_, at, soft):
            def run():
                if p_ % 2 == 0:
                    zpair[s] = pp_zf.tile([P, 2, 512], F32, tag="zf",
                                          name=f"zp{s}_{p_}")
                zp = zpair[s]
                xvs = soft.holder["xvs"]
                for c in range(LC):
                    for i in range(2):
                        nc.tensor.matmul(
                            zp[i * D:(i + 1) * D, p_ % 2, 0:QP],
                            xvs[:, c, i],
                            at[:, c, i],
                            start=(c == 0),
                            stop=(c == LC - 1),
                            skip_group_check=True,
                        )
                if s == 1 and p_ >= NP - 2:
                    # final pairs: unpaired casts so the tail fc's last
                    # contraction chunks unblock as early as possible
                    nc.vector.tensor_copy(
                        ZT[s][:, p_:p_ + 1, :], zp[:, p_ % 2:p_ % 2 + 1, 0:QP])
                elif p_ % 2 == 1:
                    nc.vector.tensor_copy(
                        ZT[s][:, p_ - 1:p_ + 1, :], zp[:, :, 0:QP])
            return run

        # ---- DMA block prefetch. The first two pairs of each sentence get
        # single-pair transfers so the pipeline head isn't stuck behind
        # megabyte bulk loads in the round-robin DMA packet scheduler. ----
        blocks = [(s, 2 * b, 2) for s in range(S)
                  for b in range(NP // 2)]
        kq_tiles = {}

        def issue_block(bi):
            if bi >= len(blocks):
                return
            s, lo, n = blocks[bi]
            xkt = kqpool.tile([P, n, L], BF16, tag=f"xk{n}", name=f"xk{s}_{lo}")
            nc.sync.dma_start(xkt[:], xk[s, lo:lo + n].rearrange("t p l -> p t l"))
            yqt = kqpool.tile([P, n, QP], BF16, tag=f"yq{n}", name=f"yq{s}_{lo}")
            nc.sync.dma_start(yqt[:], yq[s, lo:lo + n].rearrange("t p l -> p t l"))
            # the block's slice of the values tensor, issued from the gp
            # queue so the sync engine's ~700ns/issue rate doesn't delay
            # the key/query streams at kernel start
            c0, c1 = 2 * lo * D, 2 * (lo + n) * D
            nc.gpsimd.dma_start(xv_sb[s][:, :, c0:c1], xv[s, :, :, c0:c1])
            for t in range(n):
                kq_tiles[(s, lo + t)] = (xkt, yqt, t)

        # ---- main schedule ----
        rsum_gate = [None]
        xv_sb[0] = xvpool.tile([P, LC, E], BF16, tag="xv0", name="xv_0")
        xv_sb[1] = xvpool.tile([P, LC, E], BF16, tag="xv1", name="xv_1")
        issue_block(0)
        issue_block(1)
        nc.sync.dma_start(padq_sb[:], padq[:])
        issued = [2]

        def prefetch(s, p_):
            # keep blocks issued ~2 blocks (4 pairs) ahead of consumption
            if p_ % 2 == 0:
                issue_block((s * NP + p_) // 2 + 2)
                issued[0] = (s * NP + p_) // 2 + 3

        for s in range(S):
            ZT[s] = ztpool.tile([P, NP, QP], BF16, tag=f"zt{s}", name=f"zt_{s}")
            outs_sb[s] = outpool.tile([P, E // P, QP], BF16, tag=f"osb{s}",
                                      name=f"osb_{s}")

            for p_ in range(NP):
                if s == 0 and p_ == 3:
                    # big constant load once the first blocks are through.
                    # DMA issues don't block on data and Tile hoists
                    # dependency-free instructions, so gate the DMA behind a
                    # REAL data dependency: a copy that consumes pair 1's
                    # rowsum (ready only once the pipeline is flowing).
                    nc.gpsimd.tensor_copy(wcomb_sb[:, 0, 0:1],
                                          rsum_gate[0][:, 0:1])
                    nc.scalar.dma_start(wcomb_sb[:], wcomb[:])
                prefetch(s, p_)
                xkt, yqt, t = kq_tiles[(s, p_)]
                xk_p, yq_p = xkt[:, t], yqt[:, t]

                accum_mode = p_ in ACC_PAIRS

                at = atpool.tile([P, LC, 2, QP], BF16, tag="at", name=f"at{s}_{p_}")
                rsum = sumpool.tile([P, LC * 2], F32, tag="rsum", name=f"rs{s}_{p_}")
                if s == 0 and p_ == 1:
                    rsum_gate[0] = rsum

                # PE energies + ScalarE exp, chunk by chunk (2-bank tiles)
                for c in range(LC):
                    ep = pp_e.tile([P, 2, 512], F32, tag="ep",
                                   name=f"ep{s}_{p_}_{c}")
                    if s == 0:
                        for _ in range(S0_DUMMY):
                            nc.tensor.matmul(ep[:, 0, 0:QP], wsrc[0:D, 0:P],
                                             wsrc[0:D, :], start=True,
                                             stop=True)
                    for i in range(2):
                        nc.tensor.matmul(
                            ep[:, i, 0:QP],
                            xk_p[i * D:(i + 1) * D, c * P:(c + 1) * P],
                            yq_p[i * D:(i + 1) * D, :],
                            start=True,
                            stop=True,
                        )
                    if accum_mode:
                        for i in range(2):
                            nc.scalar.activation(
                                at[:, c, i], ep[:, i, 0:QP],
                                mybir.ActivationFunctionType.Exp,
                                scale=SCALE,
                                accum_out=rsum[:, c * 2 + i:c * 2 + i + 1])
                    else:
                        nc.scalar.activation(
                            at[:, c], ep[:, :, 0:QP],
                            mybir.ActivationFunctionType.Exp, scale=SCALE)

                # drain one pipelined softmax stage from the previous pair
                soft = make_soft(s, p_, at, rsum, accum_mode)
                pend_soft.append(soft)
                if len(pend_soft) > 1:
                    flush_one(pend_soft)

                # drain one pipelined attn@v from two pairs back (before
                # the fc emission: fc(s0) reads ALL of ZT[0], whose last
                # writers are these deferred casts)
                pend_attnv.append(make_attnv(s, p_, at, soft))
                if len(pend_attnv) > 2:
                    flush_one(pend_attnv)

                # fc interleave for the previous sentence
                if s == 1 and p_ >= 1:
                    emit_fc_jt(0, p_ - 1)

        while pend_soft:
            flush_one(pend_soft)
        # attnv(s1,p7) waits ~2us for its softmax chain; slot fc(0,7)
        # between the two flushes so the PE has work during that wait
        flush_one(pend_attnv)
        emit_fc_jt(0, E // P - 1)
        while pend_attnv:
            flush_one(pend_attnv)

        # fc of sentence 1 drains in the tail: the energy banks are free
        # now, so run ALL 8 jt accumulation chains across 8 PSUM banks,
        # emitted eo-major so no chain's wait on the final ZT casts blocks
        # another chain in the strict PE FIFO.
        tf = [pp_e.tile([P, 2, 512], F32, tag="ep", name=f"tf{q}")
              for q in range(2)]
        tf += [pp_zf.tile([P, 2, 512], F32, tag="zf", name=f"tf{q + 2}")
               for q in range(2)]
        for eo in range(E // P):
            for jt in range(E // P):
                nc.tensor.matmul(
                    tf[jt // 2][:, jt % 2, 0:QP],
                    wcomb_sb[:, eo, jt * P:(jt + 1) * P],
                    ZT[1][:, eo, :],
                    start=(eo == 0),
                    stop=(eo == E // P - 1),
                    skip_group_check=True,
                )
        for q in range(4):
            nc.vector.tensor_copy(
                outs_sb[1][:, 2 * q:2 * q + 2, :], tf[q][:, :, 0:QP])
            nc.sync.dma_start(outT[1, :, 2 * q:2 * q + 2, :],
                              outs_sb[1][:, 2 * q:2 * q + 2, :])


def host_prepare(values, keys, query, mask, Wv, Wk, Wq, Wo, bo):
    """Host-side sharding + layout + query compaction + weight folding."""
    values = np.asarray(values, dtype=np.float32)
    keys = np.asarray(keys, dtype=np.float32)
    query = np.asarray(query, dtype=np.float32)
    mask = np.asarray(mask)
    Wv = np.asarray(Wv, dtype=np.float32)
    Wk = np.asarray(Wk, dtype=np.float32)
    Wq = np.asarray(Wq, dtype=np.float32)
    Wo = np.asarray(Wo, dtype=np.float32)
    bo_np = np.ascontiguousarray(np.asarray(bo, dtype=np.float32))

    keep = mask[:, :, :, 0] != 0  # (N, S, L) True = query position survives
    cnt = keep.sum(-1)  # (N, S)
    # multiple of 32 so the tree-rowsum halvings stay 4-byte aligned
    QP = int(np.ceil(max(int(cnt.max()), 32) / 32) * 32)
    QP = min(QP, L)
    order = np.argsort(~keep, axis=-1, kind="stable")  # (N, S, L)

    qT = query.transpose(0, 1, 3, 2).reshape(N, S, H, D, L)
    kT = keys.transpose(0, 1, 3, 2).reshape(N, S, H, D, L)

    # gather+pad queries: (N, S, H, D, QP)
    gidx = order[:, :, :QP]  # (N, S, QP)
    qTc = np.take_along_axis(
        qT, gidx[:, :, None, None, :].repeat(H, 2).repeat(D, 3), axis=4)
    pad = np.arange(QP)[None, None, :] >= cnt[:, :, None]  # (N, S, QP)
    qTc[pad[:, :, None, None, :].repeat(H, 2).repeat(D, 3)] = 0.0

    # host q-projection: energy[q,k] = (xq A) . xk with A = Wq^T Wk
    A_T = (Wq.T @ Wk).T.copy()  # (D, D)
    yq = np.einsum("de,nshel->nshdl", A_T, qTc)  # (N, S, H, D, QP)
    yq = np.ascontiguousarray(yq.reshape(N, S, NP, 2 * D, QP))
    xkp = np.ascontiguousarray(kT.reshape(N, S, NP, 2 * D, L))

    # values pre-arranged [p, lc, e] with l = lc*128 + p
    xvp = np.ascontiguousarray(
        values.reshape(N, S, LC, P, E).transpose(0, 1, 3, 2, 4))

    wcomb = np.zeros((E, E), np.float32)
    for h in range(H):
        wcomb[h * D:(h + 1) * D, :] = Wv.T @ Wo[:, h * D:(h + 1) * D].T
    wcombp = np.ascontiguousarray(
        wcomb.reshape(E // P, P, E).transpose(1, 0, 2))

    # (N, 128, S): per-core pad-column count, replicated over partitions
    padq = np.repeat((QP - cnt).astype(np.float32)[:, None, :], P, axis=1)
    padq = np.ascontiguousarray(padq)

    import ml_dtypes
    bf = ml_dtypes.bfloat16
    yq = np.ascontiguousarray(yq.astype(bf))
    xkp = np.ascontiguousarray(xkp.astype(bf))
    xvp = np.ascontiguousarray(xvp.astype(bf))
    wcombp = np.ascontiguousarray(wcombp.astype(bf))

    in_maps = []
    for n in range(NCORES):
        m = {
            "yq": yq[n], "xk": xkp[n], "xv": xvp[n],
            "wcomb": wcombp, "padq": padq[n],
        }
        in_maps.append(m)
    return in_maps, QP, order, cnt, bo_np


_NC_CACHE = {}


def _get_program(QP):
    nc = _NC_CACHE.get(QP)
    if nc is not None:
        return nc
    nc = bacc.Bacc("TRN2", target_bir_lowering=False, debug=False,
                   num_devices=NCORES)
    ins = {
        "yq": nc.dram_tensor("yq", (S, NP, P, QP), BF16, kind="ExternalInput").ap(),
        "xk": nc.dram_tensor("xk", (S, NP, P, L), BF16, kind="ExternalInput").ap(),
        "xv": nc.dram_tensor("xv", (S, P, LC, E), BF16, kind="ExternalInput").ap(),
        "wcomb": nc.dram_tensor("wcomb", (P, E // P, E), BF16, kind="ExternalInput").ap(),
        "padq": nc.dram_tensor("padq", (P, S), F32, kind="ExternalInput").ap(),
    }
    outs = {
        "outT": nc.dram_tensor("outT", (S, P, E // P, QP), BF16,
                               kind="ExternalOutput").ap(),
    }
    with tile.TileContext(nc) as tc:
        build_kernel_body(tc, outs, ins, QP)
    nc.compile()
    _NC_CACHE[QP] = nc
    return nc


def run(inputs: dict, trace: bool = False):
    """Run on 8 cores; returns (full_output, BassKernelResults)."""
    in_maps, QP, order, cnt, bo_np = host_prepare(**inputs)
    nc = _get_program(QP)
    res = bass_utils.run_bass_kernel_spmd(
        nc, in_maps, core_ids=list(range(NCORES)), trace=trace,
    )
    out = np.empty((N, S, L, E), np.float32)
    out[:] = bo_np  # masked query rows: attention output is 0, fc adds bo
    for n in range(NCORES):
        oT = np.asarray(res.results[n]["outT"], dtype=np.float32)  # (S,P,E//P,QP)
        for s in range(S):
            c = int(cnt[n, s])
            if c:
                # oT[s, p, jt, q] -> full[e = jt*128 + p, q]
                full = oT[s].transpose(1, 0, 2).reshape(E, QP)
                out[n, s, order[n, s, :c], :] = full[:, :c].T + bo_np
    return out, res


def kernel(**inputs) -> np.ndarray:
    out, _ = run(inputs, trace=False)
    return out


# revision 57
# speedup vs baseline: 1.1774x; 1.1774x over previous
"""Trainium2 Bass kernel for nn_EntailmentSelfAttention (8-core data parallel).

Problem (per batch element n, sentence s):
  q/k/v head projections (shared per-head weights), energy = q @ k.T per head,
  query-position masking, softmax over the QUERY axis, out = attn @ v,
  fc_out: out @ Wo.T + bo.

Design (one batch element n per NeuronCore; S=2 sentences inside):
  - Transposed on-chip layout: head-dim on partitions, sequence on the free
    axis, so the softmax-over-queries reduces along the free axis.
  - q projection folded on the host (yq = q @ Wq^T Wk, compacted+padded);
    v projection folded into fc_out on the host (wcomb); fc bias on host.
  - Masked queries dropped by host compaction; pad columns carry yq = 0 so
    exp(0) = 1, and the denominator subtracts (QP - cnt) (per-core input).
  - Head PAIRS packed into 128 partitions: energy matmuls row-tiled
    (concurrent via tile_position row groups), attn@v matmuls column-tiled
    into halves of one PSUM bank.
  - exp on ScalarE reads 2-tile batches ([128, 2, QP]) from a double-buffered
    2-bank PSUM pool so ScalarE (the bottleneck engine, ~47us of exp) rarely
    starves.
  - rowsums: per pair, a 3-stage tree on the DVE (two bf16 2x-mode
    tensor_tensor halvings + one short 1x reduce) ~30% cheaper than a flat
    reduce; ACC_PAIRS pairs can ride ScalarE's accum_out instead (balance
    knob, currently empty).
  - v-scaling and denominator pad-correction on GpSimd; PSUM->SBUF casts
    for z and fc are paired (two banks per DVE cast) to amortize overhead.
  - Software pipeline: slot p emits energies(p)+exp(p), softmax(p-1),
    attn@v(p-2), fc-interleave(prev sentence, one jt per slot) so engine
    queues stay mostly dependency-free at their heads and the PE stays
    dense; a warm-up matmul burst during the DMA ramp lifts the HAM clock
    gate to full rate before real work arrives.
  - DMA: key/query streams in 2-pair blocks prefetched 2 blocks ahead from
    the sync queue; values sliced per block on the gpsimd queue; the 2MB
    wcomb load is gated behind a data dependency on pair 1's rowsum so the
    round-robin DMA packet scheduler can't starve the pipeline head with it.
"""

import math

import numpy as np

import concourse.bass as bass
import concourse.tile as tile
from concourse import bacc, mybir
from concourse import bass_utils

# problem shapes (hardcoded per the harness contract)
N, S, L, E, H = 8, 2, 512, 1024, 16
D = E // H  # 64
P = 128
NCORES = 8
LC = L // P  # 4 l-chunks
NP = H // 2  # 8 head pairs
SCALE = 1.0 / math.sqrt(float(L))

F32 = mybir.dt.float32
BF16 = mybir.dt.bfloat16

# --- tunables -------------------------------------------------------------
ACC_PAIRS = set()    # per-sentence pairs whose rowsum rides ScalarE accum_out
                     # (measured: any non-empty set regresses — the extra
                     # unbatched-exp time on ScalarE exceeds the DVE relief)
GP_XVS = True        # v-scaling multiply on GpSimd (else DVE)
WARMUP_MM = 16       # dummy matmuls at start to lift HAM to full clock
                     # during the DMA ramp; 16 ~= the 3.4us the HAM needs,
                     # ending just as the first key/query block lands
S0_DUMMY = 0         # dummy matmuls per energy chunk in sentence 0 (they
                     # delay the critical energy path more than the HAM
                     # warmth they buy; keep 0)


def build_kernel_body(tc, outs, ins, QP):
    nc = tc.nc

    def _c(ap):
        return ap if ap.dtype == BF16 else ap.bitcast(BF16)

    xk, yq, xv = _c(ins["xk"]), _c(ins["yq"]), _c(ins["xv"])
    wcomb, padq = _c(ins["wcomb"]), ins["padq"]
    outT = outs["outT"]

    import contextlib

    with contextlib.ExitStack() as ctx:
        ek = ctx.enter_context
        consts = ek(tc.tile_pool(name="consts", bufs=1))
        xvpool = ek(tc.tile_pool(name="xv", bufs=2))
        kqpool = ek(tc.tile_pool(name="kq", bufs=3))
        atpool = ek(tc.tile_pool(name="at", bufs=4))
        rtpool = ek(tc.tile_pool(name="rt", bufs=2))
        xvspool = ek(tc.tile_pool(name="xvs", bufs=3))
        sumpool = ek(tc.tile_pool(name="sums", bufs=3))
        ztpool = ek(tc.tile_pool(name="zt", bufs=2))
        outpool = ek(tc.tile_pool(name="out", bufs=2))
        pp_e = ek(tc.tile_pool(name="pp_e", bufs=2, space="PSUM"))
        pp_zf = ek(tc.tile_pool(name="pp_zf", bufs=2, space="PSUM"))

        # prime the exp ACT table load (overlaps the input DMAs)
        prim = consts.tile([P, 2], F32, tag="prim")
        nc.vector.memset(prim[:, 0:1], 0.0)
        nc.scalar.activation(prim[:, 1:2], prim[:, 0:1],
                             mybir.ActivationFunctionType.Exp)

        # PE warm-up burst: dense dummy matmuls during the DMA ramp lift the
        # HAM clock-gate to 8/8; steady-state gaps stay below the ~3.4us MID
        # window, so the PE then runs at full clock for the whole kernel.
        wsrc = consts.tile([P, QP], BF16, tag="wsrc")
        nc.gpsimd.memset(wsrc[:], 0.0)
        if WARMUP_MM:
            pwu = pp_e.tile([P, 2, 512], F32, tag="ep", name="ep_warm")
            for w in range(WARMUP_MM):
                nc.tensor.matmul(pwu[:, w % 2, 0:QP], wsrc[0:P, 0:P],
                                 wsrc[:], start=True, stop=True)

        padq_sb = consts.tile([P, S], F32, tag="padq")
        wcomb_sb = consts.tile([P, E // P, E], BF16, tag="wcomb")

        xv_sb, ZT, outs_sb = {}, {}, {}
        pfpair = [None]

        def emit_fc_jt(s, jt):
            if jt % 2 == 0:
                pfpair[0] = pp_zf.tile([P, 2, 512], F32, tag="zf",
                                       name=f"pf_{s}_{jt}")
            pf = pfpair[0]
            for eo in range(E // P):
                nc.tensor.matmul(
                    pf[:, jt % 2, 0:QP],
                    wcomb_sb[:, eo, jt * P:(jt + 1) * P],
                    ZT[s][:, eo, :],
                    start=(eo == 0),
                    stop=(eo == E // P - 1),
                )
            if jt % 2 == 1:
                nc.vector.tensor_copy(
                    outs_sb[s][:, jt - 1:jt + 1, :], pf[:, :, 0:QP])
                nc.sync.dma_start(outT[s, :, jt - 1:jt + 1, :],
                                  outs_sb[s][:, jt - 1:jt + 1, :])

        # ---- deferred (software-pipelined) stage emitters ----
        pend_attnv = []
        pend_soft = []
        zpair = {}

        def flush_one(q):
            if q:
                q.pop(0)()

        def make_soft(s, p_, at, rsum, accum_mode):
            holder = {}

            def run():
                if not accum_mode:
                    # 3-stage tree rowsum: the two TT-add stages run at the
                    # DVE's 2x bf16 mode, leaving only a short 1x reduce
                    h1 = QP // 2
                    h2 = QP // 4
                    r1 = rtpool.tile([P, LC, 2, h1], BF16, tag="r1",
                                     name=f"r1{s}_{p_}")
                    nc.vector.tensor_tensor(
                        r1[:], at[:, :, :, 0:h1], at[:, :, :, h1:QP],
                        mybir.AluOpType.add)
                    r2 = rtpool.tile([P, LC, 2, h2], BF16, tag="r2",
                                     name=f"r2{s}_{p_}")
                    nc.vector.tensor_tensor(
                        r2[:], r1[:, :, :, 0:h2], r1[:, :, :, h2:h1],
                        mybir.AluOpType.add)
                    nc.vector.tensor_reduce(
                        rsum[:].rearrange("p (c i) -> p c i", i=2),
                        r2[:],
                        axis=mybir.AxisListType.X,
                        op=mybir.AluOpType.add,
                    )
                den = sumpool.tile([P, LC * 2], F32, tag="den", name=f"dn{s}_{p_}")
                nc.gpsimd.tensor_tensor(
                    den[:], rsum[:],
                    padq_sb[:, s:s + 1].to_broadcast((P, LC * 2)),
                    mybir.AluOpType.subtract)
                recip = sumpool.tile([P, LC, 2], F32, tag="recip", name=f"rc{s}_{p_}")
                nc.vector.reciprocal(recip[:].rearrange("p c i -> p (c i)"), den[:])
                xvs = xvspool.tile([P, LC, 2, D], BF16, tag="xvs", name=f"xs{s}_{p_}")
                xv_view = xv_sb[s][:, :, 2 * p_ * D:(2 * p_ + 2) * D].rearrange(
                    "p c (i d) -> p c i d", d=D)
                eng = nc.gpsimd if GP_XVS else nc.vector
                eng.tensor_tensor(
                    xvs[:], xv_view,
                    recip[:, :, :, None].to_broadcast((P, LC, 2, D)),
                    mybir.AluOpType.mult)
                holder["xvs"] = xvs
            run.holder = holder
            return run

        def make_attnv(s, p_, at, soft):
            def run():
                if p_ % 2 == 0:
                    zpair[s] = pp_zf.tile([P, 2, 512], F32, tag="zf",
                                          name=f"zp{s}_{p_}")
                zp = zpair[s]
                xvs = soft.holder["xvs"]
                for c in range(LC):
                    for i in range(2):
                        nc.tensor.matmul(
                            zp[i * D:(i + 1) * D, p_ % 2, 0:QP],
                            xvs[:, c, i],
                            at[:, c, i],
                            start=(c == 0),
                            stop=(c == LC - 1),
                            skip_group_check=True,
                        )
                if p_ % 2 == 1:
                    nc.vector.tensor_copy(
                        ZT[s][:, p_ - 1:p_ + 1, :], zp[:, :, 0:QP])
            return run

        # ---- DMA block prefetch. The first two pairs of each sentence get
        # single-pair transfers so the pipeline head isn't stuck behind
        # megabyte bulk loads in the round-robin DMA packet scheduler. ----
        blocks = [(s, 2 * b, 2) for s in range(S)
                  for b in range(NP // 2)]
        kq_tiles = {}

        def issue_block(bi):
            if bi >= len(blocks):
                return
            s, lo, n = blocks[bi]
            xkt = kqpool.tile([P, n, L], BF16, tag=f"xk{n}", name=f"xk{s}_{lo}")
            nc.sync.dma_start(xkt[:], xk[s, lo:lo + n].rearrange("t p l -> p t l"))
            yqt = kqpool.tile([P, n, QP], BF16, tag=f"yq{n}", name=f"yq{s}_{lo}")
            nc.sync.dma_start(yqt[:], yq[s, lo:lo + n].rearrange("t p l -> p t l"))
            # the block's slice of the values tensor, issued from the gp
            # queue so the sync engine's ~700ns/issue rate doesn't delay
            # the key/query streams at kernel start
            c0, c1 = 2 * lo * D, 2 * (lo + n) * D
            nc.gpsimd.dma_start(xv_sb[s][:, :, c0:c1], xv[s, :, :, c0:c1])
            for t in range(n):
                kq_tiles[(s, lo + t)] = (xkt, yqt, t)

        # ---- main schedule ----
        rsum_gate = [None]
        xv_sb[0] = xvpool.tile([P, LC, E], BF16, tag="xv0", name="xv_0")
        xv_sb[1] = xvpool.tile([P, LC, E], BF16, tag="xv1", name="xv_1")
        issue_block(0)
        issue_block(1)
        nc.sync.dma_start(padq_sb[:], padq[:])
        issued = [2]

        def prefetch(s, p_):
            # keep blocks issued ~2 blocks (4 pairs) ahead of consumption
            if p_ % 2 == 0:
                issue_block((s * NP + p_) // 2 + 2)
                issued[0] = (s * NP + p_) // 2 + 3

        for s in range(S):
            ZT[s] = ztpool.tile([P, NP, QP], BF16, tag=f"zt{s}", name=f"zt_{s}")
            outs_sb[s] = outpool.tile([P, E // P, QP], BF16, tag=f"osb{s}",
                                      name=f"osb_{s}")

            for p_ in range(NP):
                if s == 0 and p_ == 3:
                    # big constant load once the first blocks are through.
                    # DMA issues don't block on data and Tile hoists
                    # dependency-free instructions, so gate the DMA behind a
                    # REAL data dependency: a copy that consumes pair 1's
                    # rowsum (ready only once the pipeline is flowing).
                    nc.gpsimd.tensor_copy(wcomb_sb[:, 0, 0:1],
                                          rsum_gate[0][:, 0:1])
                    nc.scalar.dma_start(wcomb_sb[:], wcomb[:])
                prefetch(s, p_)
                xkt, yqt, t = kq_tiles[(s, p_)]
                xk_p, yq_p = xkt[:, t], yqt[:, t]

                accum_mode = p_ in ACC_PAIRS

                at = atpool.tile([P, LC, 2, QP], BF16, tag="at", name=f"at{s}_{p_}")
                rsum = sumpool.tile([P, LC * 2], F32, tag="rsum", name=f"rs{s}_{p_}")
                if s == 0 and p_ == 1:
                    rsum_gate[0] = rsum

                # PE energies + ScalarE exp, chunk by chunk (2-bank tiles)
                for c in range(LC):
                    ep = pp_e.tile([P, 2, 512], F32, tag="ep",
                                   name=f"ep{s}_{p_}_{c}")
                    if s == 0:
                        for _ in range(S0_DUMMY):
                            nc.tensor.matmul(ep[:, 0, 0:QP], wsrc[0:D, 0:P],
                                             wsrc[0:D, :], start=True,
                                             stop=True)
                    for i in range(2):
                        nc.tensor.matmul(
                            ep[:, i, 0:QP],
                            xk_p[i * D:(i + 1) * D, c * P:(c + 1) * P],
                            yq_p[i * D:(i + 1) * D, :],
                            start=True,
                            stop=True,
                        )
                    if accum_mode:
                        for i in range(2):
                            nc.scalar.activation(
                                at[:, c, i], ep[:, i, 0:QP],
                                mybir.ActivationFunctionType.Exp,
                                scale=SCALE,
                                accum_out=rsum[:, c * 2 + i:c * 2 + i + 1])
                    else:
                        nc.scalar.activation(
                            at[:, c], ep[:, :, 0:QP],
                            mybir.ActivationFunctionType.Exp, scale=SCALE)

                # drain one pipelined softmax stage from the previous pair
                soft = make_soft(s, p_, at, rsum, accum_mode)
                pend_soft.append(soft)
                if len(pend_soft) > 1:
                    flush_one(pend_soft)

                # drain one pipelined attn@v from two pairs back (before
                # the fc emission: fc(s0) reads ALL of ZT[0], whose last
                # writers are these deferred casts)
                pend_attnv.append(make_attnv(s, p_, at, soft))
                if len(pend_attnv) > 2:
                    flush_one(pend_attnv)

                # fc interleave for the previous sentence
                if s == 1 and p_ >= 1:
                    emit_fc_jt(0, p_ - 1)

        while pend_soft:
            flush_one(pend_soft)
        # attnv(s1,p7) waits ~2us for its softmax chain; slot fc(0,7)
        # between the two flushes so the PE has work during that wait
        flush_one(pend_attnv)
        emit_fc_jt(0, E // P - 1)
        while pend_attnv:
            flush_one(pend_attnv)
        for jt in range(E // P):
            emit_fc_jt(1, jt)


def host_prepare(values, keys, query, mask, Wv, Wk, Wq, Wo, bo):
    """Host-side sharding + layout + query compaction + weight folding."""
    values = np.asarray(values, dtype=np.float32)
    keys = np.asarray(keys, dtype=np.float32)
    query = np.asarray(query, dtype=np.float32)
    mask = np.asarray(mask)
    Wv = np.asarray(Wv, dtype=np.float32)
    Wk = np.asarray(Wk, dtype=np.float32)
    Wq = np.asarray(Wq, dtype=np.float32)
    Wo = np.asarray(Wo, dtype=np.float32)
    bo_np = np.ascontiguousarray(np.asarray(bo, dtype=np.float32))

    keep = mask[:, :, :, 0] != 0  # (N, S, L) True = query position survives
    cnt = keep.sum(-1)  # (N, S)
    # multiple of 32 so the tree-rowsum halvings stay 4-byte aligned
    QP = int(np.ceil(max(int(cnt.max()), 32) / 32) * 32)
    QP = min(QP, L)
    order = np.argsort(~keep, axis=-1, kind="stable")  # (N, S, L)

    qT = query.transpose(0, 1, 3, 2).reshape(N, S, H, D, L)
    kT = keys.transpose(0, 1, 3, 2).reshape(N, S, H, D, L)

    # gather+pad queries: (N, S, H, D, QP)
    gidx = order[:, :, :QP]  # (N, S, QP)
    qTc = np.take_along_axis(
        qT, gidx[:, :, None, None, :].repeat(H, 2).repeat(D, 3), axis=4)
    pad = np.arange(QP)[None, None, :] >= cnt[:, :, None]  # (N, S, QP)
    qTc[pad[:, :, None, None, :].repeat(H, 2).repeat(D, 3)] = 0.0

    # host q-projection: energy[q,k] = (xq A) . xk with A = Wq^T Wk
    A_T = (Wq.T @ Wk).T.copy()  # (D, D)
    yq = np.einsum("de,nshel->nshdl", A_T, qTc)  # (N, S, H, D, QP)
    yq = np.ascontiguousarray(yq.reshape(N, S, NP, 2 * D, QP))
    xkp = np.ascontiguousarray(kT.reshape(N, S, NP, 2 * D, L))

    # values pre-arranged [p, lc, e] with l = lc*128 + p
    xvp = np.ascontiguousarray(
        values.reshape(N, S, LC, P, E).transpose(0, 1, 3, 2, 4))

    wcomb = np.zeros((E, E), np.float32)
    for h in range(H):
        wcomb[h * D:(h + 1) * D, :] = Wv.T @ Wo[:, h * D:(h + 1) * D].T
    wcombp = np.ascontiguousarray(
        wcomb.reshape(E // P, P, E).transpose(1, 0, 2))

    # (N, 128, S): per-core pad-column count, replicated over partitions
    padq = np.repeat((QP - cnt).astype(np.float32)[:, None, :], P, axis=1)
    padq = np.ascontiguousarray(padq)

    import ml_dtypes
    bf = ml_dtypes.bfloat16
    yq = np.ascontiguousarray(yq.astype(bf))
    xkp = np.ascontiguousarray(xkp.astype(bf))
    xvp = np.ascontiguousarray(xvp.astype(bf))
    wcombp = np.ascontiguousarray(wcombp.astype(bf))

    in_maps = []
    for n in range(NCORES):
        m = {
            "yq": yq[n], "xk": xkp[n], "xv": xvp[n],
            "wcomb": wcombp, "padq": padq[n],
        }
        in_maps.append(m)
    return in_maps, QP, order, cnt, bo_np


_NC_CACHE = {}


def _get_program(QP):
    nc = _NC_CACHE.get(QP)
    if nc is not None:
        return nc
    nc = bacc.Bacc("TRN2", target_bir_lowering=False, debug=False,
                   num_devices=NCORES)
    ins = {
        "yq": nc.dram_tensor("yq", (S, NP, P, QP), BF16, kind="ExternalInput").ap(),
        "xk": nc.dram_tensor("xk", (S, NP, P, L), BF16, kind="ExternalInput").ap(),
        "xv": nc.dram_tensor("xv", (S, P, LC, E), BF16, kind="ExternalInput").ap(),
        "wcomb": nc.dram_tensor("wcomb", (P, E // P, E), BF16, kind="ExternalInput").ap(),
        "padq": nc.dram_tensor("padq", (P, S), F32, kind="ExternalInput").ap(),
    }
    outs = {
        "outT": nc.dram_tensor("outT", (S, P, E // P, QP), BF16,
                               kind="ExternalOutput").ap(),
    }
    with tile.TileContext(nc) as tc:
        build_kernel_body(tc, outs, ins, QP)
    nc.compile()
    _NC_CACHE[QP] = nc
    return nc


def run(inputs: dict, trace: bool = False):
    """Run on 8 cores; returns (full_output, BassKernelResults)."""
    in_maps, QP, order, cnt, bo_np = host_prepare(**inputs)
    nc = _get_program(QP)
    res = bass_utils.run_bass_kernel_spmd(
        nc, in_maps, core_ids=list(range(NCORES)), trace=trace,
    )
    out = np.empty((N, S, L, E), np.float32)
    out[:] = bo_np  # masked query rows: attention output is 0, fc adds bo
    for n in range(NCORES):
        oT = np.asarray(res.results[n]["outT"], dtype=np.float32)  # (S,P,E//P,QP)
        for s in range(S):
            c = int(cnt[n, s])
            if c:
                # oT[s, p, jt, q] -> full[e = jt*128 + p, q]
                full = oT[s].transpose(1, 0, 2).reshape(E, QP)
                out[n, s, order[n, s, :c], :] = full[:, :c].T + bo_np
    return out, res


def kernel(**inputs) -> np.ndarray:
    out, _ = run(inputs, trace=False)
    return out


# revision 59
# speedup vs baseline: 1.2129x; 1.0301x over previous
"""Trainium2 Bass kernel for nn_EntailmentSelfAttention (8-core data parallel).

Problem (per batch element n, sentence s):
  q/k/v head projections (shared per-head weights), energy = q @ k.T per head,
  query-position masking, softmax over the QUERY axis, out = attn @ v,
  fc_out: out @ Wo.T + bo.

Design (one batch element n per NeuronCore; S=2 sentences inside):
  - Transposed on-chip layout: head-dim on partitions, sequence on the free
    axis, so the softmax-over-queries reduces along the free axis.
  - q projection folded on the host (yq = q @ Wq^T Wk, compacted+padded);
    v projection folded into fc_out on the host (wcomb); fc bias on host.
  - Masked queries dropped by host compaction; pad columns carry yq = 0 so
    exp(0) = 1, and the denominator subtracts (QP - cnt) (per-core input).
  - Head PAIRS packed into 128 partitions: energy matmuls row-tiled
    (concurrent via tile_position row groups), attn@v matmuls column-tiled
    into halves of one PSUM bank.
  - exp on ScalarE reads 2-tile batches ([128, 2, QP]) from a double-buffered
    2-bank PSUM pool so ScalarE (the bottleneck engine, ~47us of exp) rarely
    starves.
  - rowsums: per pair, a 3-stage tree on the DVE (two bf16 2x-mode
    tensor_tensor halvings + one short 1x reduce) ~30% cheaper than a flat
    reduce; ACC_PAIRS pairs can ride ScalarE's accum_out instead (balance
    knob, currently empty).
  - v-scaling and denominator pad-correction on GpSimd; PSUM->SBUF casts
    for z and fc are paired (two banks per DVE cast) to amortize overhead.
  - Software pipeline: slot p emits energies(p)+exp(p), softmax(p-1),
    attn@v(p-2), fc-interleave(prev sentence, one jt per slot) so engine
    queues stay mostly dependency-free at their heads and the PE stays
    dense; a warm-up matmul burst during the DMA ramp lifts the HAM clock
    gate to full rate before real work arrives.
  - DMA: key/query streams in 2-pair blocks prefetched 2 blocks ahead from
    the sync queue; values sliced per block on the gpsimd queue; the 2MB
    wcomb load is gated behind a data dependency on pair 1's rowsum so the
    round-robin DMA packet scheduler can't starve the pipeline head with it.
"""

import math

import numpy as np

import concourse.bass as bass
import concourse.tile as tile
from concourse import bacc, mybir
from concourse import bass_utils

# problem shapes (hardcoded per the harness contract)
N, S, L, E, H = 8, 2, 512, 1024, 16
D = E // H  # 64
P = 128
NCORES = 8
LC = L // P  # 4 l-chunks
NP = H // 2  # 8 head pairs
SCALE = 1.0 / math.sqrt(float(L))

F32 = mybir.dt.float32
BF16 = mybir.dt.bfloat16

# --- tunables -------------------------------------------------------------
ACC_PAIRS = set()    # per-sentence pairs whose rowsum rides ScalarE accum_out
                     # (measured: any non-empty set regresses — the extra
                     # unbatched-exp time on ScalarE exceeds the DVE relief)
GP_XVS = False       # v-scaling multiply on GpSimd (else DVE). False keeps
                     # the whole tree->sub->recip->xvs chain on the DVE:
                     # no cross-engine hops, so xvs lands earlier and the
                     # attn@v -> energy -> exp chain stalls less
WARMUP_MM = 16       # dummy matmuls at start to lift HAM to full clock
                     # during the DMA ramp; 16 ~= the 3.4us the HAM needs,
                     # ending just as the first key/query block lands
S0_DUMMY = 0         # dummy matmuls per energy chunk in sentence 0 (they
                     # delay the critical energy path more than the HAM
                     # warmth they buy; keep 0)


def build_kernel_body(tc, outs, ins, QP):
    nc = tc.nc

    def _c(ap):
        return ap if ap.dtype == BF16 else ap.bitcast(BF16)

    xk, yq, xv = _c(ins["xk"]), _c(ins["yq"]), _c(ins["xv"])
    wcomb, padq = _c(ins["wcomb"]), ins["padq"]
    outT = outs["outT"]

    import contextlib

    with contextlib.ExitStack() as ctx:
        ek = ctx.enter_context
        consts = ek(tc.tile_pool(name="consts", bufs=1))
        xvpool = ek(tc.tile_pool(name="xv", bufs=2))
        kqpool = ek(tc.tile_pool(name="kq", bufs=3))
        atpool = ek(tc.tile_pool(name="at", bufs=4))
        rtpool = ek(tc.tile_pool(name="rt", bufs=2))
        xvspool = ek(tc.tile_pool(name="xvs", bufs=3))
        sumpool = ek(tc.tile_pool(name="sums", bufs=3))
        ztpool = ek(tc.tile_pool(name="zt", bufs=2))
        outpool = ek(tc.tile_pool(name="out", bufs=2))
        pp_e = ek(tc.tile_pool(name="pp_e", bufs=2, space="PSUM"))
        pp_zf = ek(tc.tile_pool(name="pp_zf", bufs=2, space="PSUM"))

        # prime the exp ACT table load (overlaps the input DMAs)
        prim = consts.tile([P, 2], F32, tag="prim")
        nc.vector.memset(prim[:, 0:1], 0.0)
        nc.scalar.activation(prim[:, 1:2], prim[:, 0:1],
                             mybir.ActivationFunctionType.Exp)

        # PE warm-up burst: dense dummy matmuls during the DMA ramp lift the
        # HAM clock-gate to 8/8; steady-state gaps stay below the ~3.4us MID
        # window, so the PE then runs at full clock for the whole kernel.
        wsrc = consts.tile([P, QP], BF16, tag="wsrc")
        nc.gpsimd.memset(wsrc[:], 0.0)
        if WARMUP_MM:
            pwu = pp_e.tile([P, 2, 512], F32, tag="ep", name="ep_warm")
            for w in range(WARMUP_MM):
                nc.tensor.matmul(pwu[:, w % 2, 0:QP], wsrc[0:P, 0:P],
                                 wsrc[:], start=True, stop=True)

        padq_sb = consts.tile([P, S], F32, tag="padq")
        wcomb_sb = consts.tile([P, E // P, E], BF16, tag="wcomb")

        xv_sb, ZT, outs_sb = {}, {}, {}
        pfpair = [None]

        def emit_fc_jt(s, jt):
            if jt % 2 == 0:
                pfpair[0] = pp_zf.tile([P, 2, 512], F32, tag="zf",
                                       name=f"pf_{s}_{jt}")
            pf = pfpair[0]
            for eo in range(E // P):
                nc.tensor.matmul(
                    pf[:, jt % 2, 0:QP],
                    wcomb_sb[:, eo, jt * P:(jt + 1) * P],
                    ZT[s][:, eo, :],
                    start=(eo == 0),
                    stop=(eo == E // P - 1),
                )
            if jt % 2 == 1:
                nc.vector.tensor_copy(
                    outs_sb[s][:, jt - 1:jt + 1, :], pf[:, :, 0:QP])
                nc.sync.dma_start(outT[s, :, jt - 1:jt + 1, :],
                                  outs_sb[s][:, jt - 1:jt + 1, :])

        # ---- deferred (software-pipelined) stage emitters ----
        pend_attnv = []
        pend_soft = []
        zpair = {}

        def flush_one(q):
            if q:
                q.pop(0)()

        def make_soft(s, p_, at, rsum, accum_mode):
            holder = {}

            def run():
                if not accum_mode:
                    # 3-stage tree rowsum: the two TT-add stages run at the
                    # DVE's 2x bf16 mode, leaving only a short 1x reduce
                    h1 = QP // 2
                    h2 = QP // 4
                    r1 = rtpool.tile([P, LC, 2, h1], BF16, tag="r1",
                                     name=f"r1{s}_{p_}")
                    nc.vector.tensor_tensor(
                        r1[:], at[:, :, :, 0:h1], at[:, :, :, h1:QP],
                        mybir.AluOpType.add)
                    r2 = rtpool.tile([P, LC, 2, h2], BF16, tag="r2",
                                     name=f"r2{s}_{p_}")
                    nc.vector.tensor_tensor(
                        r2[:], r1[:, :, :, 0:h2], r1[:, :, :, h2:h1],
                        mybir.AluOpType.add)
                    nc.vector.tensor_reduce(
                        rsum[:].rearrange("p (c i) -> p c i", i=2),
                        r2[:],
                        axis=mybir.AxisListType.X,
                        op=mybir.AluOpType.add,
                    )
                den = sumpool.tile([P, LC * 2], F32, tag="den", name=f"dn{s}_{p_}")
                nc.vector.tensor_tensor(
                    den[:], rsum[:],
                    padq_sb[:, s:s + 1].to_broadcast((P, LC * 2)),
                    mybir.AluOpType.subtract)
                recip = sumpool.tile([P, LC, 2], F32, tag="recip", name=f"rc{s}_{p_}")
                nc.vector.reciprocal(recip[:].rearrange("p c i -> p (c i)"), den[:])
                xvs = xvspool.tile([P, LC, 2, D], BF16, tag="xvs", name=f"xs{s}_{p_}")
                xv_view = xv_sb[s][:, :, 2 * p_ * D:(2 * p_ + 2) * D].rearrange(
                    "p c (i d) -> p c i d", d=D)
                eng = nc.gpsimd if GP_XVS else nc.vector
                eng.tensor_tensor(
                    xvs[:], xv_view,
                    recip[:, :, :, None].to_broadcast((P, LC, 2, D)),
                    mybir.AluOpType.mult)
                holder["xvs"] = xvs
            run.holder = holder
            return run

        def make_attnv(s, p_, at, soft):
            def run():
                if p_ % 2 == 0:
                    zpair[s] = pp_zf.tile([P, 2, 512], F32, tag="zf",
                                          name=f"zp{s}_{p_}")
                zp = zpair[s]
                xvs = soft.holder["xvs"]
                for c in range(LC):
                    for i in range(2):
                        nc.tensor.matmul(
                            zp[i * D:(i + 1) * D, p_ % 2, 0:QP],
                            xvs[:, c, i],
                            at[:, c, i],
                            start=(c == 0),
                            stop=(c == LC - 1),
                            skip_group_check=True,
                        )
                if p_ % 2 == 1:
                    nc.vector.tensor_copy(
                        ZT[s][:, p_ - 1:p_ + 1, :], zp[:, :, 0:QP])
            return run

        # ---- DMA block prefetch. The first two pairs of each sentence get
        # single-pair transfers so the pipeline head isn't stuck behind
        # megabyte bulk loads in the round-robin DMA packet scheduler. ----
        blocks = [(s, 2 * b, 2) for s in range(S)
                  for b in range(NP // 2)]
        kq_tiles = {}

        def issue_block(bi):
            if bi >= len(blocks):
                return
            s, lo, n = blocks[bi]
            xkt = kqpool.tile([P, n, L], BF16, tag=f"xk{n}", name=f"xk{s}_{lo}")
            nc.sync.dma_start(xkt[:], xk[s, lo:lo + n].rearrange("t p l -> p t l"))
            yqt = kqpool.tile([P, n, QP], BF16, tag=f"yq{n}", name=f"yq{s}_{lo}")
            nc.sync.dma_start(yqt[:], yq[s, lo:lo + n].rearrange("t p l -> p t l"))
            # the block's slice of the values tensor, issued from the gp
            # queue so the sync engine's ~700ns/issue rate doesn't delay
            # the key/query streams at kernel start
            c0, c1 = 2 * lo * D, 2 * (lo + n) * D
            nc.gpsimd.dma_start(xv_sb[s][:, :, c0:c1], xv[s, :, :, c0:c1])
            for t in range(n):
                kq_tiles[(s, lo + t)] = (xkt, yqt, t)

        # ---- main schedule ----
        rsum_gate = [None]
        xv_sb[0] = xvpool.tile([P, LC, E], BF16, tag="xv0", name="xv_0")
        xv_sb[1] = xvpool.tile([P, LC, E], BF16, tag="xv1", name="xv_1")
        issue_block(0)
        issue_block(1)
        nc.sync.dma_start(padq_sb[:], padq[:])
        issued = [2]

        def prefetch(s, p_):
            # keep blocks issued ~2 blocks (4 pairs) ahead of consumption
            if p_ % 2 == 0:
                issue_block((s * NP + p_) // 2 + 2)
                issued[0] = (s * NP + p_) // 2 + 3

        for s in range(S):
            ZT[s] = ztpool.tile([P, NP, QP], BF16, tag=f"zt{s}", name=f"zt_{s}")
            outs_sb[s] = outpool.tile([P, E // P, QP], BF16, tag=f"osb{s}",
                                      name=f"osb_{s}")

            for p_ in range(NP):
                if s == 0 and p_ == 3:
                    # big constant load once the first blocks are through.
                    # DMA issues don't block on data and Tile hoists
                    # dependency-free instructions, so gate the DMA behind a
                    # REAL data dependency: a copy that consumes pair 1's
                    # rowsum (ready only once the pipeline is flowing).
                    nc.gpsimd.tensor_copy(wcomb_sb[:, 0, 0:1],
                                          rsum_gate[0][:, 0:1])
                    nc.scalar.dma_start(wcomb_sb[:], wcomb[:])
                prefetch(s, p_)
                xkt, yqt, t = kq_tiles[(s, p_)]
                xk_p, yq_p = xkt[:, t], yqt[:, t]

                accum_mode = p_ in ACC_PAIRS

                at = atpool.tile([P, LC, 2, QP], BF16, tag="at", name=f"at{s}_{p_}")
                rsum = sumpool.tile([P, LC * 2], F32, tag="rsum", name=f"rs{s}_{p_}")
                if s == 0 and p_ == 1:
                    rsum_gate[0] = rsum

                # PE energies + ScalarE exp, chunk by chunk (2-bank tiles)
                for c in range(LC):
                    ep = pp_e.tile([P, 2, 512], F32, tag="ep",
                                   name=f"ep{s}_{p_}_{c}")
                    if s == 0:
                        for _ in range(S0_DUMMY):
                            nc.tensor.matmul(ep[:, 0, 0:QP], wsrc[0:D, 0:P],
                                             wsrc[0:D, :], start=True,
                                             stop=True)
                    for i in range(2):
                        nc.tensor.matmul(
                            ep[:, i, 0:QP],
                            xk_p[i * D:(i + 1) * D, c * P:(c + 1) * P],
                            yq_p[i * D:(i + 1) * D, :],
                            start=True,
                            stop=True,
                        )
                    if accum_mode:
                        for i in range(2):
                            nc.scalar.activation(
                                at[:, c, i], ep[:, i, 0:QP],
                                mybir.ActivationFunctionType.Exp,
                                scale=SCALE,
                                accum_out=rsum[:, c * 2 + i:c * 2 + i + 1])
                    else:
                        nc.scalar.activation(
                            at[:, c], ep[:, :, 0:QP],
                            mybir.ActivationFunctionType.Exp, scale=SCALE)

                # drain one pipelined softmax stage from the previous pair
                soft = make_soft(s, p_, at, rsum, accum_mode)
                pend_soft.append(soft)
                if len(pend_soft) > 1:
                    flush_one(pend_soft)

                # drain one pipelined attn@v from two pairs back (before
                # the fc emission: fc(s0) reads ALL of ZT[0], whose last
                # writers are these deferred casts)
                pend_attnv.append(make_attnv(s, p_, at, soft))
                if len(pend_attnv) > 2:
                    flush_one(pend_attnv)

                # fc interleave for the previous sentence
                if s == 1 and p_ >= 1:
                    emit_fc_jt(0, p_ - 1)

        while pend_soft:
            flush_one(pend_soft)
        # attnv(s1,p7) waits ~2us for its softmax chain; slot fc(0,7)
        # between the two flushes so the PE has work during that wait
        flush_one(pend_attnv)
        emit_fc_jt(0, E // P - 1)
        while pend_attnv:
            flush_one(pend_attnv)
        for jt in range(E // P):
            emit_fc_jt(1, jt)


def host_prepare(values, keys, query, mask, Wv, Wk, Wq, Wo, bo):
    """Host-side sharding + layout + query compaction + weight folding."""
    values = np.asarray(values, dtype=np.float32)
    keys = np.asarray(keys, dtype=np.float32)
    query = np.asarray(query, dtype=np.float32)
    mask = np.asarray(mask)
    Wv = np.asarray(Wv, dtype=np.float32)
    Wk = np.asarray(Wk, dtype=np.float32)
    Wq = np.asarray(Wq, dtype=np.float32)
    Wo = np.asarray(Wo, dtype=np.float32)
    bo_np = np.ascontiguousarray(np.asarray(bo, dtype=np.float32))

    keep = mask[:, :, :, 0] != 0  # (N, S, L) True = query position survives
    cnt = keep.sum(-1)  # (N, S)
    # multiple of 32 so the tree-rowsum halvings stay 4-byte aligned
    QP = int(np.ceil(max(int(cnt.max()), 32) / 32) * 32)
    QP = min(QP, L)
    order = np.argsort(~keep, axis=-1, kind="stable")  # (N, S, L)

    qT = query.transpose(0, 1, 3, 2).reshape(N, S, H, D, L)
    kT = keys.transpose(0, 1, 3, 2).reshape(N, S, H, D, L)

    # gather+pad queries: (N, S, H, D, QP)
    gidx = order[:, :, :QP]  # (N, S, QP)
    qTc = np.take_along_axis(
        qT, gidx[:, :, None, None, :].repeat(H, 2).repeat(D, 3), axis=4)
    pad = np.arange(QP)[None, None, :] >= cnt[:, :, None]  # (N, S, QP)
    qTc[pad[:, :, None, None, :].repeat(H, 2).repeat(D, 3)] = 0.0

    # host q-projection: energy[q,k] = (xq A) . xk with A = Wq^T Wk
    A_T = (Wq.T @ Wk).T.copy()  # (D, D)
    yq = np.einsum("de,nshel->nshdl", A_T, qTc)  # (N, S, H, D, QP)
    yq = np.ascontiguousarray(yq.reshape(N, S, NP, 2 * D, QP))
    xkp = np.ascontiguousarray(kT.reshape(N, S, NP, 2 * D, L))

    # values pre-arranged [p, lc, e] with l = lc*128 + p
    xvp = np.ascontiguousarray(
        values.reshape(N, S, LC, P, E).transpose(0, 1, 3, 2, 4))

    wcomb = np.zeros((E, E), np.float32)
    for h in range(H):
        wcomb[h * D:(h + 1) * D, :] = Wv.T @ Wo[:, h * D:(h + 1) * D].T
    wcombp = np.ascontiguousarray(
        wcomb.reshape(E // P, P, E).transpose(1, 0, 2))

    # (N, 128, S): per-core pad-column count, replicated over partitions
    padq = np.repeat((QP - cnt).astype(np.float32)[:, None, :], P, axis=1)
    padq = np.ascontiguousarray(padq)

    import ml_dtypes
    bf = ml_dtypes.bfloat16
    yq = np.ascontiguousarray(yq.astype(bf))
    xkp = np.ascontiguousarray(xkp.astype(bf))
    xvp = np.ascontiguousarray(xvp.astype(bf))
    wcombp = np.ascontiguousarray(wcombp.astype(bf))

    in_maps = []
    for n in range(NCORES):
        m = {
            "yq": yq[n], "xk": xkp[n], "xv": xvp[n],
            "wcomb": wcombp, "padq": padq[n],
        }
        in_maps.append(m)
    return in_maps, QP, order, cnt, bo_np


_NC_CACHE = {}


def _get_program(QP):
    nc = _NC_CACHE.get(QP)
    if nc is not None:
        return nc
    nc = bacc.Bacc("TRN2", target_bir_lowering=False, debug=False,
                   num_devices=NCORES)
    ins = {
        "yq": nc.dram_tensor("yq", (S, NP, P, QP), BF16, kind="ExternalInput").ap(),
        "xk": nc.dram_tensor("xk", (S, NP, P, L), BF16, kind="ExternalInput").ap(),
        "xv": nc.dram_tensor("xv", (S, P, LC, E), BF16, kind="ExternalInput").ap(),
        "wcomb": nc.dram_tensor("wcomb", (P, E // P, E), BF16, kind="ExternalInput").ap(),
        "padq": nc.dram_tensor("padq", (P, S), F32, kind="ExternalInput").ap(),
    }
    outs = {
        "outT": nc.dram_tensor("outT", (S, P, E // P, QP), BF16,
                               kind="ExternalOutput").ap(),
    }
    with tile.TileContext(nc) as tc:
        build_kernel_body(tc, outs, ins, QP)
    nc.compile()
    _NC_CACHE[QP] = nc
    return nc


def run(inputs: dict, trace: bool = False):
    """Run on 8 cores; returns (full_output, BassKernelResults)."""
    in_maps, QP, order, cnt, bo_np = host_prepare(**inputs)
    nc = _get_program(QP)
    res = bass_utils.run_bass_kernel_spmd(
        nc, in_maps, core_ids=list(range(NCORES)), trace=trace,
    )
    out = np.empty((N, S, L, E), np.float32)
    out[:] = bo_np  # masked query rows: attention output is 0, fc adds bo
    for n in range(NCORES):
        oT = np.asarray(res.results[n]["outT"], dtype=np.float32)  # (S,P,E//P,QP)
        for s in range(S):
            c = int(cnt[n, s])
            if c:
                # oT[s, p, jt, q] -> full[e = jt*128 + p, q]
                full = oT[s].transpose(1, 0, 2).reshape(E, QP)
                out[n, s, order[n, s, :c], :] = full[:, :c].T + bo_np
    return out, res


def kernel(**inputs) -> np.ndarray:
    out, _ = run(inputs, trace=False)
    return out


# revision 60
# speedup vs baseline: 1.2941x; 1.0669x over previous
"""Trainium2 Bass kernel for nn_EntailmentSelfAttention (8-core data parallel).

Problem (per batch element n, sentence s):
  q/k/v head projections (shared per-head weights), energy = q @ k.T per head,
  query-position masking, softmax over the QUERY axis, out = attn @ v,
  fc_out: out @ Wo.T + bo.

Design (one batch element n per NeuronCore; S=2 sentences inside):
  - Transposed on-chip layout: head-dim on partitions, sequence on the free
    axis, so the softmax-over-queries reduces along the free axis.
  - q projection folded on the host (yq = q @ Wq^T Wk, compacted+padded);
    v projection folded into fc_out on the host (wcomb); fc bias on host.
  - Masked queries dropped by host compaction; pad columns carry yq = 0 so
    exp(0) = 1, and the denominator subtracts (QP - cnt) (per-core input).
  - Head PAIRS packed into 128 partitions: energy matmuls row-tiled
    (concurrent via tile_position row groups), attn@v matmuls column-tiled
    into halves of one PSUM bank.
  - exp on ScalarE reads 2-tile batches ([128, 2, QP]) from a double-buffered
    2-bank PSUM pool so ScalarE (the bottleneck engine, ~47us of exp) rarely
    starves.
  - rowsums: per pair, a 3-stage tree on the DVE (two bf16 2x-mode
    tensor_tensor halvings + one short 1x reduce) ~30% cheaper than a flat
    reduce; ACC_PAIRS pairs can ride ScalarE's accum_out instead (balance
    knob, currently empty).
  - v-scaling and denominator pad-correction on GpSimd; PSUM->SBUF casts
    for z and fc are paired (two banks per DVE cast) to amortize overhead.
  - Software pipeline: slot p emits energies(p)+exp(p), softmax(p-1),
    attn@v(p-2), fc-interleave(prev sentence, one jt per slot) so engine
    queues stay mostly dependency-free at their heads and the PE stays
    dense; a warm-up matmul burst during the DMA ramp lifts the HAM clock
    gate to full rate before real work arrives.
  - DMA: key/query streams in 2-pair blocks prefetched 2 blocks ahead from
    the sync queue; values sliced per block on the gpsimd queue; the 2MB
    wcomb load is gated behind a data dependency on pair 1's rowsum so the
    round-robin DMA packet scheduler can't starve the pipeline head with it.
"""

import math

import numpy as np

import concourse.bass as bass
import concourse.tile as tile
from concourse import bacc, mybir
from concourse import bass_utils

# problem shapes (hardcoded per the harness contract)
N, S, L, E, H = 8, 2, 512, 1024, 16
D = E // H  # 64
P = 128
NCORES = 8
LC = L // P  # 4 l-chunks
NP = H // 2  # 8 head pairs
SCALE = 1.0 / math.sqrt(float(L))

F32 = mybir.dt.float32
BF16 = mybir.dt.bfloat16

# --- tunables -------------------------------------------------------------
ACC_PAIRS = set()    # per-sentence pairs whose rowsum rides ScalarE accum_out
                     # (measured: any non-empty set regresses — the extra
                     # unbatched-exp time on ScalarE exceeds the DVE relief)
GP_XVS = False       # v-scaling multiply on GpSimd (else DVE). False keeps
                     # the whole tree->sub->recip->xvs chain on the DVE:
                     # no cross-engine hops, so xvs lands earlier and the
                     # attn@v -> energy -> exp chain stalls less
WARMUP_MM = 16       # dummy matmuls at start to lift HAM to full clock
                     # during the DMA ramp; 16 ~= the 3.4us the HAM needs,
                     # ending just as the first key/query block lands
S0_DUMMY = 0         # dummy matmuls per energy chunk in sentence 0 (they
                     # delay the critical energy path more than the HAM
                     # warmth they buy; keep 0)


def build_kernel_body(tc, outs, ins, QP):
    nc = tc.nc

    def _c(ap):
        return ap if ap.dtype == BF16 else ap.bitcast(BF16)

    xk, yq, xv = _c(ins["xk"]), _c(ins["yq"]), _c(ins["xv"])
    wcomb, padq = _c(ins["wcomb"]), ins["padq"]
    outT = outs["outT"]

    import contextlib

    with contextlib.ExitStack() as ctx:
        ek = ctx.enter_context
        consts = ek(tc.tile_pool(name="consts", bufs=1))
        xvpool = ek(tc.tile_pool(name="xv", bufs=2))
        kqpool = ek(tc.tile_pool(name="kq", bufs=3))
        atpool = ek(tc.tile_pool(name="at", bufs=4))
        rtpool = ek(tc.tile_pool(name="rt", bufs=2))
        xvspool = ek(tc.tile_pool(name="xvs", bufs=3))
        sumpool = ek(tc.tile_pool(name="sums", bufs=3))
        ztpool = ek(tc.tile_pool(name="zt", bufs=2))
        outpool = ek(tc.tile_pool(name="out", bufs=2))
        pp_e = ek(tc.tile_pool(name="pp_e", bufs=2, space="PSUM"))
        pp_zf = ek(tc.tile_pool(name="pp_zf", bufs=2, space="PSUM"))

        # prime the exp ACT table load (overlaps the input DMAs)
        prim = consts.tile([P, 2], F32, tag="prim")
        nc.vector.memset(prim[:, 0:1], 0.0)
        nc.scalar.activation(prim[:, 1:2], prim[:, 0:1],
                             mybir.ActivationFunctionType.Exp)

        # PE warm-up burst: dense dummy matmuls during the DMA ramp lift the
        # HAM clock-gate to 8/8; steady-state gaps stay below the ~3.4us MID
        # window, so the PE then runs at full clock for the whole kernel.
        wsrc = consts.tile([P, QP], BF16, tag="wsrc")
        nc.gpsimd.memset(wsrc[:], 0.0)
        if WARMUP_MM:
            pwu = pp_e.tile([P, 2, 512], F32, tag="ep", name="ep_warm")
            for w in range(WARMUP_MM):
                nc.tensor.matmul(pwu[:, w % 2, 0:QP], wsrc[0:P, 0:P],
                                 wsrc[:], start=True, stop=True)

        padq_sb = consts.tile([P, S], F32, tag="padq")
        wcomb_sb = consts.tile([P, E // P, E], BF16, tag="wcomb")

        xv_sb, ZT, outs_sb = {}, {}, {}
        pfpair = [None]

        def emit_fc_jt(s, jt):
            if jt % 2 == 0:
                pfpair[0] = pp_zf.tile([P, 2, 512], F32, tag="zf",
                                       name=f"pf_{s}_{jt}")
            pf = pfpair[0]
            for eo in range(E // P):
                nc.tensor.matmul(
                    pf[:, jt % 2, 0:QP],
                    wcomb_sb[:, eo, jt * P:(jt + 1) * P],
                    ZT[s][:, eo, :],
                    start=(eo == 0),
                    stop=(eo == E // P - 1),
                )
            if jt % 2 == 1:
                # evacuate on ScalarE: the DVE is the busiest elementwise
                # engine and these casts are off the exp-critical chain,
                # while ScalarE has slack in the inter-slot gaps
                nc.scalar.copy(
                    outs_sb[s][:, jt - 1:jt + 1, :], pf[:, :, 0:QP])
                nc.sync.dma_start(outT[s, :, jt - 1:jt + 1, :],
                                  outs_sb[s][:, jt - 1:jt + 1, :])

        # ---- deferred (software-pipelined) stage emitters ----
        pend_attnv = []
        pend_soft = []
        zpair = {}

        def flush_one(q):
            if q:
                q.pop(0)()

        def make_soft(s, p_, at, rsum, accum_mode):
            holder = {}

            def run():
                if not accum_mode:
                    # 3-stage tree rowsum: the two TT-add stages run at the
                    # DVE's 2x bf16 mode, leaving only a short 1x reduce
                    h1 = QP // 2
                    h2 = QP // 4
                    r1 = rtpool.tile([P, LC, 2, h1], BF16, tag="r1",
                                     name=f"r1{s}_{p_}")
                    nc.vector.tensor_tensor(
                        r1[:], at[:, :, :, 0:h1], at[:, :, :, h1:QP],
                        mybir.AluOpType.add)
                    r2 = rtpool.tile([P, LC, 2, h2], BF16, tag="r2",
                                     name=f"r2{s}_{p_}")
                    nc.vector.tensor_tensor(
                        r2[:], r1[:, :, :, 0:h2], r1[:, :, :, h2:h1],
                        mybir.AluOpType.add)
                    nc.vector.tensor_reduce(
                        rsum[:].rearrange("p (c i) -> p c i", i=2),
                        r2[:],
                        axis=mybir.AxisListType.X,
                        op=mybir.AluOpType.add,
                    )
                den = sumpool.tile([P, LC * 2], F32, tag="den", name=f"dn{s}_{p_}")
                nc.vector.tensor_tensor(
                    den[:], rsum[:],
                    padq_sb[:, s:s + 1].to_broadcast((P, LC * 2)),
                    mybir.AluOpType.subtract)
                recip = sumpool.tile([P, LC, 2], F32, tag="recip", name=f"rc{s}_{p_}")
                nc.vector.reciprocal(recip[:].rearrange("p c i -> p (c i)"), den[:])
                xvs = xvspool.tile([P, LC, 2, D], BF16, tag="xvs", name=f"xs{s}_{p_}")
                xv_view = xv_sb[s][:, :, 2 * p_ * D:(2 * p_ + 2) * D].rearrange(
                    "p c (i d) -> p c i d", d=D)
                eng = nc.gpsimd if GP_XVS else nc.vector
                eng.tensor_tensor(
                    xvs[:], xv_view,
                    recip[:, :, :, None].to_broadcast((P, LC, 2, D)),
                    mybir.AluOpType.mult)
                holder["xvs"] = xvs
            run.holder = holder
            return run

        def make_attnv(s, p_, at, soft):
            def run():
                if p_ % 2 == 0:
                    zpair[s] = pp_zf.tile([P, 2, 512], F32, tag="zf",
                                          name=f"zp{s}_{p_}")
                zp = zpair[s]
                xvs = soft.holder["xvs"]
                for c in range(LC):
                    for i in range(2):
                        nc.tensor.matmul(
                            zp[i * D:(i + 1) * D, p_ % 2, 0:QP],
                            xvs[:, c, i],
                            at[:, c, i],
                            start=(c == 0),
                            stop=(c == LC - 1),
                            skip_group_check=True,
                        )
                if p_ % 2 == 1:
                    nc.vector.tensor_copy(
                        ZT[s][:, p_ - 1:p_ + 1, :], zp[:, :, 0:QP])
            return run

        # ---- DMA block prefetch. The first two pairs of each sentence get
        # single-pair transfers so the pipeline head isn't stuck behind
        # megabyte bulk loads in the round-robin DMA packet scheduler. ----
        blocks = [(s, 2 * b, 2) for s in range(S)
                  for b in range(NP // 2)]
        kq_tiles = {}

        def issue_block(bi):
            if bi >= len(blocks):
                return
            s, lo, n = blocks[bi]
            xkt = kqpool.tile([P, n, L], BF16, tag=f"xk{n}", name=f"xk{s}_{lo}")
            nc.sync.dma_start(xkt[:], xk[s, lo:lo + n].rearrange("t p l -> p t l"))
            yqt = kqpool.tile([P, n, QP], BF16, tag=f"yq{n}", name=f"yq{s}_{lo}")
            nc.sync.dma_start(yqt[:], yq[s, lo:lo + n].rearrange("t p l -> p t l"))
            # the block's slice of the values tensor, issued from the gp
            # queue so the sync engine's ~700ns/issue rate doesn't delay
            # the key/query streams at kernel start
            c0, c1 = 2 * lo * D, 2 * (lo + n) * D
            nc.gpsimd.dma_start(xv_sb[s][:, :, c0:c1], xv[s, :, :, c0:c1])
            for t in range(n):
                kq_tiles[(s, lo + t)] = (xkt, yqt, t)

        # ---- main schedule ----
        rsum_gate = [None]
        xv_sb[0] = xvpool.tile([P, LC, E], BF16, tag="xv0", name="xv_0")
        xv_sb[1] = xvpool.tile([P, LC, E], BF16, tag="xv1", name="xv_1")
        issue_block(0)
        issue_block(1)
        nc.sync.dma_start(padq_sb[:], padq[:])
        issued = [2]

        def prefetch(s, p_):
            # keep blocks issued ~2 blocks (4 pairs) ahead of consumption
            if p_ % 2 == 0:
                issue_block((s * NP + p_) // 2 + 2)
                issued[0] = (s * NP + p_) // 2 + 3

        for s in range(S):
            ZT[s] = ztpool.tile([P, NP, QP], BF16, tag=f"zt{s}", name=f"zt_{s}")
            outs_sb[s] = outpool.tile([P, E // P, QP], BF16, tag=f"osb{s}",
                                      name=f"osb_{s}")

            for p_ in range(NP):
                if s == 0 and p_ == 3:
                    # big constant load once the first blocks are through.
                    # DMA issues don't block on data and Tile hoists
                    # dependency-free instructions, so gate the DMA behind a
                    # REAL data dependency: a copy that consumes pair 1's
                    # rowsum (ready only once the pipeline is flowing).
                    nc.gpsimd.tensor_copy(wcomb_sb[:, 0, 0:1],
                                          rsum_gate[0][:, 0:1])
                    nc.scalar.dma_start(wcomb_sb[:], wcomb[:])
                prefetch(s, p_)
                xkt, yqt, t = kq_tiles[(s, p_)]
                xk_p, yq_p = xkt[:, t], yqt[:, t]

                accum_mode = p_ in ACC_PAIRS

                at = atpool.tile([P, LC, 2, QP], BF16, tag="at", name=f"at{s}_{p_}")
                rsum = sumpool.tile([P, LC * 2], F32, tag="rsum", name=f"rs{s}_{p_}")
                if s == 0 and p_ == 1:
                    rsum_gate[0] = rsum

                # PE energies + ScalarE exp, chunk by chunk (2-bank tiles)
                for c in range(LC):
                    ep = pp_e.tile([P, 2, 512], F32, tag="ep",
                                   name=f"ep{s}_{p_}_{c}")
                    if s == 0:
                        for _ in range(S0_DUMMY):
                            nc.tensor.matmul(ep[:, 0, 0:QP], wsrc[0:D, 0:P],
                                             wsrc[0:D, :], start=True,
                                             stop=True)
                    for i in range(2):
                        nc.tensor.matmul(
                            ep[:, i, 0:QP],
                            xk_p[i * D:(i + 1) * D, c * P:(c + 1) * P],
                            yq_p[i * D:(i + 1) * D, :],
                            start=True,
                            stop=True,
                        )
                    if accum_mode:
                        for i in range(2):
                            nc.scalar.activation(
                                at[:, c, i], ep[:, i, 0:QP],
                                mybir.ActivationFunctionType.Exp,
                                scale=SCALE,
                                accum_out=rsum[:, c * 2 + i:c * 2 + i + 1])
                    else:
                        nc.scalar.activation(
                            at[:, c], ep[:, :, 0:QP],
                            mybir.ActivationFunctionType.Exp, scale=SCALE)

                # drain one pipelined softmax stage from the previous pair
                soft = make_soft(s, p_, at, rsum, accum_mode)
                pend_soft.append(soft)
                if len(pend_soft) > 1:
                    flush_one(pend_soft)

                # drain one pipelined attn@v from two pairs back (before
                # the fc emission: fc(s0) reads ALL of ZT[0], whose last
                # writers are these deferred casts)
                pend_attnv.append(make_attnv(s, p_, at, soft))
                if len(pend_attnv) > 2:
                    flush_one(pend_attnv)

                # fc interleave for the previous sentence
                if s == 1 and p_ >= 1:
                    emit_fc_jt(0, p_ - 1)

        while pend_soft:
            flush_one(pend_soft)
        # attnv(s1,p7) waits ~2us for its softmax chain; slot fc(0,7)
        # between the two flushes so the PE has work during that wait
        flush_one(pend_attnv)
        emit_fc_jt(0, E // P - 1)
        while pend_attnv:
            flush_one(pend_attnv)
        for jt in range(E // P):
            emit_fc_jt(1, jt)


def host_prepare(values, keys, query, mask, Wv, Wk, Wq, Wo, bo):
    """Host-side sharding + layout + query compaction + weight folding."""
    values = np.asarray(values, dtype=np.float32)
    keys = np.asarray(keys, dtype=np.float32)
    query = np.asarray(query, dtype=np.float32)
    mask = np.asarray(mask)
    Wv = np.asarray(Wv, dtype=np.float32)
    Wk = np.asarray(Wk, dtype=np.float32)
    Wq = np.asarray(Wq, dtype=np.float32)
    Wo = np.asarray(Wo, dtype=np.float32)
    bo_np = np.ascontiguousarray(np.asarray(bo, dtype=np.float32))

    keep = mask[:, :, :, 0] != 0  # (N, S, L) True = query position survives
    cnt = keep.sum(-1)  # (N, S)
    # multiple of 32 so the tree-rowsum halvings stay 4-byte aligned
    QP = int(np.ceil(max(int(cnt.max()), 32) / 32) * 32)
    QP = min(QP, L)
    order = np.argsort(~keep, axis=-1, kind="stable")  # (N, S, L)

    qT = query.transpose(0, 1, 3, 2).reshape(N, S, H, D, L)
    kT = keys.transpose(0, 1, 3, 2).reshape(N, S, H, D, L)

    # gather+pad queries: (N, S, H, D, QP)
    gidx = order[:, :, :QP]  # (N, S, QP)
    qTc = np.take_along_axis(
        qT, gidx[:, :, None, None, :].repeat(H, 2).repeat(D, 3), axis=4)
    pad = np.arange(QP)[None, None, :] >= cnt[:, :, None]  # (N, S, QP)
    qTc[pad[:, :, None, None, :].repeat(H, 2).repeat(D, 3)] = 0.0

    # host q-projection: energy[q,k] = (xq A) . xk with A = Wq^T Wk
    A_T = (Wq.T @ Wk).T.copy()  # (D, D)
    yq = np.einsum("de,nshel->nshdl", A_T, qTc)  # (N, S, H, D, QP)
    yq = np.ascontiguousarray(yq.reshape(N, S, NP, 2 * D, QP))
    xkp = np.ascontiguousarray(kT.reshape(N, S, NP, 2 * D, L))

    # values pre-arranged [p, lc, e] with l = lc*128 + p
    xvp = np.ascontiguousarray(
        values.reshape(N, S, LC, P, E).transpose(0, 1, 3, 2, 4))

    wcomb = np.zeros((E, E), np.float32)
    for h in range(H):
        wcomb[h * D:(h + 1) * D, :] = Wv.T @ Wo[:, h * D:(h + 1) * D].T
    wcombp = np.ascontiguousarray(
        wcomb.reshape(E // P, P, E).transpose(1, 0, 2))

    # (N, 128, S): per-core pad-column count, replicated over partitions
    padq = np.repeat((QP - cnt).astype(np.float32)[:, None, :], P, axis=1)
    padq = np.ascontiguousarray(padq)

    import ml_dtypes
    bf = ml_dtypes.bfloat16
    yq = np.ascontiguousarray(yq.astype(bf))
    xkp = np.ascontiguousarray(xkp.astype(bf))
    xvp = np.ascontiguousarray(xvp.astype(bf))
    wcombp = np.ascontiguousarray(wcombp.astype(bf))

    in_maps = []
    for n in range(NCORES):
        m = {
            "yq": yq[n], "xk": xkp[n], "xv": xvp[n],
            "wcomb": wcombp, "padq": padq[n],
        }
        in_maps.append(m)
    return in_maps, QP, order, cnt, bo_np


_NC_CACHE = {}


def _get_program(QP):
    nc = _NC_CACHE.get(QP)
    if nc is not None:
        return nc
    nc = bacc.Bacc("TRN2", target_bir_lowering=False, debug=False,
                   num_devices=NCORES)
    ins = {
        "yq": nc.dram_tensor("yq", (S, NP, P, QP), BF16, kind="ExternalInput").ap(),
        "xk": nc.dram_tensor("xk", (S, NP, P, L), BF16, kind="ExternalInput").ap(),
        "xv": nc.dram_tensor("xv", (S, P, LC, E), BF16, kind="ExternalInput").ap(),
        "wcomb": nc.dram_tensor("wcomb", (P, E // P, E), BF16, kind="ExternalInput").ap(),
        "padq": nc.dram_tensor("padq", (P, S), F32, kind="ExternalInput").ap(),
    }
    outs = {
        "outT": nc.dram_tensor("outT", (S, P, E // P, QP), BF16,
                               kind="ExternalOutput").ap(),
    }
    with tile.TileContext(nc) as tc:
        build_kernel_body(tc, outs, ins, QP)
    nc.compile()
    _NC_CACHE[QP] = nc
    return nc


def run(inputs: dict, trace: bool = False):
    """Run on 8 cores; returns (full_output, BassKernelResults)."""
    in_maps, QP, order, cnt, bo_np = host_prepare(**inputs)
    nc = _get_program(QP)
    res = bass_utils.run_bass_kernel_spmd(
        nc, in_maps, core_ids=list(range(NCORES)), trace=trace,
    )
    out = np.empty((N, S, L, E), np.float32)
    out[:] = bo_np  # masked query rows: attention output is 0, fc adds bo
    for n in range(NCORES):
        oT = np.asarray(res.results[n]["outT"], dtype=np.float32)  # (S,P,E//P,QP)
        for s in range(S):
            c = int(cnt[n, s])
            if c:
                # oT[s, p, jt, q] -> full[e = jt*128 + p, q]
                full = oT[s].transpose(1, 0, 2).reshape(E, QP)
                out[n, s, order[n, s, :c], :] = full[:, :c].T + bo_np
    return out, res


def kernel(**inputs) -> np.ndarray:
    out, _ = run(inputs, trace=False)
    return out
